# revision 10
# baseline (speedup 1.0000x reference)
"""BayesianLinear (y = x @ (mu + softplus(rho) * eps).T + bias) on 8 TRN2 cores.

Column-parallel sharding: each core owns OUT_F/8 = 512 output features.

Host-side prep is pure layout/precision staging (no reference math):
  - x is cast to bf16 and pre-tiled into the SBUF layout the TensorEngine
    needs for its stationary operand. Group tiles (0-7) are laid out
    k-chunk-major so ONE DMA delivers a k-chunk of all 8 tiles with
    fully-contiguous per-partition runs; extras (8-11) ship as two
    half-K blocks; stream tiles (12-63) are tile-major (1MB DMAs).
  - weight mu/rho/eps shards are transposed to [in_f, o_sh] and packed
    per single K-block into one bf16-typed tensor (mu bf16 | eps bf16 |
    rho fp16-bits): 32 DMAs of 393KB in k order on the sync queue, so
    the first W^T block is constructible ~4us after queue start. rho
    ships fp16 because softplus amplifies its quantization ~3x.

Device per core:
  1. sync queue: bias row (6KB), wpk singles k=0..31, then stream x
     tiles (5-deep prefetch ring). scalar queue: group k-chunks and
     extras halves (ordered by consumption time), then y writes. The
     two HWDGE queues round-robin at packet level so weight and x
     delivery progress together; y rides scalar where a trigger's
     eviction-wait cannot head-of-line block stream-x prefetch.
  2. W^T construction per K-block: single-op Softplus(rho) on ACT
     (bf16 out), mul(eps)/add(mu) on DVE at 2x bf16 rate into the
     resident W^T tile [128, 32, 512]. The scalar FIFO interleaves
     x-chunk triggers between softplus ops; the DVE FIFO interleaves
     construction with partial-eviction ops in consumption order
     (wt0-15, partsA, wt16-23/partsA2, wt24-31, tails) so PSUM banks
     free on time.
  3. PE program: warmup K=1 matmuls (HAM ramp + cover of first-block
     latency), bias broadcast (ones.T @ bias_bf), then split-K: tiles
     0-7 accumulate k 0..15 k-interleaved (paced against delivery),
     park bias-pre-added partials in SBUF as bf16; extras 8-11 stream
     first halves at full speed off the resident W^T; both sets finish
     k 16..31. Remaining 52 tiles stream one PSUM bank each; DVE fuses
     eviction with the partial/bias add; y out on the scalar queue.
"""

import numpy as np
import ml_dtypes

import concourse.bacc as bacc
import concourse.mybir as mybir
import concourse.tile as tile
from concourse.bass_utils import run_bass_kernel_spmd

BATCH = 8192
IN_F = 4096
OUT_F = 4096
N_CORES = 8
P = 128

GROUP = 8
EXTRA = 4
NGRP = GROUP + EXTRA
# k-chunk sizes for the group-of-8 delivery: small early chunks for low
# time-to-first-row; must sum to KB=32 with a boundary at HALF=16.
CHUNKS = [4, 4, 4, 4, 8, 8]

_NC_CACHE = {}


def build_nc(batch=BATCH, in_f=IN_F, o_sh=OUT_F // N_CORES):
    KB = in_f // P  # K-blocks of 128 along the contraction dim
    BT = batch // P  # 128-row output tiles
    HALF = KB // 2

    nc = bacc.Bacc(
        "TRN2",
        target_bir_lowering=False,
        debug=False,
        enable_asserts=False,
        num_devices=N_CORES,
    )
    bf16 = mybir.dt.bfloat16
    f16 = mybir.dt.float16
    f32 = mybir.dt.float32

    cb = [0]
    for c in CHUNKS:
        cb.append(cb[-1] + c)
    assert cb[-1] == KB and HALF in cb
    NCH = len(CHUNKS)

    xg = nc.declare_dram_parameter("x_g", [P, KB, GROUP, P], bf16, isOutput=False)
    xea = nc.declare_dram_parameter("x_ea", [P, HALF, EXTRA, P], bf16, isOutput=False)
    xeb = nc.declare_dram_parameter("x_eb", [P, HALF, EXTRA, P], bf16, isOutput=False)
    xs = nc.declare_dram_parameter("x_s", [BT - NGRP, P, KB, P], bf16, isOutput=False)
    wpk = nc.declare_dram_parameter("wpk_t", [KB, P, 3 * o_sh], bf16, isOutput=False)
    bpk = nc.declare_dram_parameter("bias_pk", [1, 3 * o_sh], f32, isOutput=False)
    y = nc.declare_dram_parameter("y", [batch, o_sh], f32, isOutput=True)

    act_exp = mybir.ActivationFunctionType.Exp
    act_ln = mybir.ActivationFunctionType.Ln

    N_WARM = 17
    RING = 3  # pk/sp construction ring depth

    with tile.TileContext(nc) as tc:
        with (
            tc.tile_pool(name="const", bufs=1) as const,
            tc.tile_pool(name="wcons", bufs=RING) as wcons,
            tc.tile_pool(name="xin", bufs=4) as xin,
            tc.tile_pool(name="part", bufs=NGRP) as part,
            tc.tile_pool(name="yout", bufs=2) as yout,
            tc.tile_pool(name="psum", bufs=8, space="PSUM") as psum_pool,
        ):
            bias_sb = const.tile([P, o_sh], f32, tag="bias_sb")
            bias_bf = const.tile([1, o_sh], bf16, tag="bias_bf")
            ones = const.tile([1, P], bf16, tag="ones")
            nc.vector.memset(ones[:], 1.0)
            wones = const.tile([1, o_sh], bf16, tag="wones")
            nc.vector.memset(wones[:], 1.0)

            # Bias inputs ride the sync queue ahead of everything (6 KiB,
            # one packed partition-0 [1, 3*o_sh] DMA: mu | rho | eps).
            b_all = const.tile([1, 3 * o_sh], f32, tag="b_all")
            nc.sync.dma_start(out=b_all[:], in_=bpk[:])
            b_mu = b_all[:, 0:o_sh]
            b_rho = b_all[:, o_sh : 2 * o_sh]
            b_eps = b_all[:, 2 * o_sh : 3 * o_sh]
            b_sp = const.tile([1, o_sh], f32, tag="b_sp")
            nc.scalar.activation(b_sp[:], b_rho, act_exp)
            nc.scalar.activation(b_sp[:], b_sp[:], act_ln, bias=1.0)
            nc.vector.tensor_mul(out=b_sp[:], in0=b_sp[:], in1=b_eps)
            nc.vector.tensor_add(out=bias_bf[:], in0=b_sp[:], in1=b_mu)

            # PE warmup + bias broadcast emitted early so the DVE-side
            # bias_sb eviction precedes all construction ops in the DVE
            # FIFO (partials add bias_sb at pass-A end).
            warm_ps = psum_pool.tile([P, o_sh], f32, tag="ps", name="warm_ps")
            for w in range(N_WARM):
                nc.tensor.matmul(warm_ps[:], lhsT=ones[:], rhs=wones[:])
            bias_ps = psum_pool.tile([P, o_sh], f32, tag="ps", name="bias_ps")
            nc.tensor.matmul(bias_ps[:], lhsT=ones[:], rhs=bias_bf[:])
            nc.vector.tensor_copy(out=bias_sb[:], in_=bias_ps[:])

            WT_bf = const.tile([P, KB, o_sh], bf16, tag="WT_bf")

            # Group x chunk tiles (k-chunk-major, all 8 tiles per chunk)
            xg_sb = [
                const.tile(
                    [P, CHUNKS[c], GROUP, P], bf16, tag=f"xg_c{c}", name=f"xg_c{c}"
                )
                for c in range(NCH)
            ]
            xe_sb = [
                const.tile(
                    [P, HALF, EXTRA, P], bf16, tag=f"xe_h{h}", name=f"xe_h{h}"
                )
                for h in range(2)
            ]

            # ---- sync-queue DMA program: wpk singles in k order (the
            # RING-deep pk ring's reuse dep gates trigger k on the DVE
            # add of k-RING, which stays ahead of queue drain).
            pks = []
            for k in range(KB):
                pk = wcons.tile([P, 3 * o_sh], bf16, tag="pk", name=f"pk{k}")
                nc.sync.dma_start(out=pk[:], in_=wpk[k])
                pks.append(pk)

            # ---- scalar engine program: group-x/extras triggers
            # interleaved with the softplus stream (strict FIFO), ordered
            # by consumption time.
            sps = []

            def emit_sp(k):
                # softplus(rho) = ln(1 + exp(rho)); Exp+Ln share one
                # ACT table set (natural_log_exp_and_others).
                pk = pks[k]
                rho_t = pk[:, 2 * o_sh : 3 * o_sh].bitcast(f16)
                sp_f = wcons.tile([P, o_sh], f16, tag="spf", name=f"spf{k}")
                sp_t = wcons.tile([P, o_sh], bf16, tag="sp", name=f"sp{k}")
                nc.scalar.activation(sp_f[:], rho_t[:], act_exp)
                nc.scalar.activation(sp_t[:], sp_f[:], act_ln, bias=1.0)
                sps.append(sp_t)

            def emit_wt(k):
                pk = pks[k]
                mu_t = pk[:, 0:o_sh]
                eps_t = pk[:, o_sh : 2 * o_sh]
                sp_t = sps[k]
                nc.vector.tensor_mul(out=sp_t[:], in0=sp_t[:], in1=eps_t[:])
                nc.vector.tensor_add(out=WT_bf[:, k, :], in0=sp_t[:], in1=mu_t[:])

            def trig_xg(c):
                nc.scalar.dma_start(out=xg_sb[c][:], in_=xg[:, cb[c] : cb[c + 1]])

            trig_xg(0)
            trig_xg(1)
            emit_sp(0)
            emit_sp(1)
            trig_xg(2)
            emit_sp(2)
            emit_sp(3)
            trig_xg(3)
            emit_sp(4)
            emit_sp(5)
            nc.scalar.dma_start(out=xe_sb[0][:], in_=xea[:])
            for k in range(6, 10):
                emit_sp(k)
            trig_xg(4)
            for k in range(10, 14):
                emit_sp(k)
            trig_xg(5)
            for k in range(14, 18):
                emit_sp(k)
            nc.scalar.dma_start(out=xe_sb[1][:], in_=xeb[:])
            for k in range(18, KB):
                emit_sp(k)

            # ---- DVE construction for the first half (pass-A weights)
            for k in range(HALF):
                emit_wt(k)

            def glhs(i, k):
                """lhsT AP for group tile i (0..NGRP-1), k-block k."""
                if i < GROUP:
                    c = next(
                        ci for ci in range(NCH) if cb[ci] <= k < cb[ci + 1]
                    )
                    return xg_sb[c][:, k - cb[c], i, :]
                return xe_sb[k // HALF][:, k % HALF, i - GROUP, :]

            # Split-K pass A: tiles 0-7, k 0..15, k-interleaved.
            pss = [
                psum_pool.tile([P, o_sh], f32, tag="ps", name=f"ps_a{bt}")
                for bt in range(GROUP)
            ]
            for k in range(HALF):
                for i in range(GROUP):
                    nc.tensor.matmul(
                        pss[i][:],
                        lhsT=glhs(i, k),
                        rhs=WT_bf[:, k, :],
                        start=(k == 0),
                        stop=(k == HALF - 1),
                    )
            parts = []
            for i in range(GROUP):
                pa = part.tile([P, o_sh], bf16, tag="pA", name=f"pA_{i}")
                nc.vector.tensor_add(out=pa[:], in0=pss[i][:], in1=bias_sb[:])
                parts.append(pa)
            # A2: extras' first halves at full speed (W^T 0..HALF
            # resident); second-half constructions interleave on DVE so
            # each partsA2 eviction stays unblocked in the FIFO.
            wt_next = HALF
            for e in range(GROUP, NGRP):
                emit_wt(wt_next)
                emit_wt(wt_next + 1)
                wt_next += 2
                ps = psum_pool.tile([P, o_sh], f32, tag="ps", name=f"ps_a{e}")
                for k in range(HALF):
                    nc.tensor.matmul(
                        ps[:],
                        lhsT=glhs(e, k),
                        rhs=WT_bf[:, k, :],
                        start=(k == 0),
                        stop=(k == HALF - 1),
                    )
                pa = part.tile([P, o_sh], bf16, tag="pA", name=f"pA_{e}")
                nc.vector.tensor_add(out=pa[:], in0=ps[:], in1=bias_sb[:])
                parts.append(pa)
            for k in range(wt_next, KB):
                emit_wt(k)
            # B: tiles 0-7 second halves (k-interleaved)
            psb = [
                psum_pool.tile([P, o_sh], f32, tag="ps", name=f"ps_b{bt}")
                for bt in range(GROUP)
            ]
            for k in range(HALF, KB):
                for i in range(GROUP):
                    nc.tensor.matmul(
                        psb[i][:],
                        lhsT=glhs(i, k),
                        rhs=WT_bf[:, k, :],
                        start=(k == HALF),
                        stop=(k == KB - 1),
                    )

            def split_tail(ps, pa, bt):
                y_sb = yout.tile([P, o_sh], f32, tag="y_sb")
                nc.vector.tensor_add(out=y_sb[:], in0=ps[:], in1=pa[:])
                nc.scalar.dma_start(out=y[bt * P : (bt + 1) * P, :], in_=y_sb[:])

            for i in range(GROUP):
                split_tail(psb[i], parts[i], i)
            # B2: extras' second halves at full speed
            for e in range(GROUP, NGRP):
                ps = psum_pool.tile([P, o_sh], f32, tag="ps", name=f"ps_b{e}")
                for k in range(HALF, KB):
                    nc.tensor.matmul(
                        ps[:],
                        lhsT=glhs(e, k),
                        rhs=WT_bf[:, k, :],
                        start=(k == HALF),
                        stop=(k == KB - 1),
                    )
                split_tail(ps, parts[e], e)

            # ---- remaining tiles stream one PSUM bank each off a
            # 5-deep prefetch ring on the sync queue.
            for bt in range(NGRP, BT):
                xbf_t = xin.tile([P, KB, P], bf16, tag="xT")
                nc.sync.dma_start(out=xbf_t[:], in_=xs[bt - NGRP])
                ps = psum_pool.tile([P, o_sh], f32, tag="ps")
                for k in range(KB):
                    nc.tensor.matmul(
                        ps[:],
                        lhsT=xbf_t[:, k, :],
                        rhs=WT_bf[:, k, :],
                        start=(k == 0),
                        stop=(k == KB - 1),
                    )
                y_sb = yout.tile([P, o_sh], f32, tag="y_sb")
                nc.vector.tensor_add(out=y_sb[:], in0=ps[:], in1=bias_sb[:])
                nc.scalar.dma_start(out=y[bt * P : (bt + 1) * P, :], in_=y_sb[:])

    # Skip bacc's pre-placed InstLoadActFuncSet: on large graphs walrus's
    # parallel-pass fork can separate the hoisted load from its activations
    # ("No Act func set exist for this instruction"); walrus's own lower_act
    # placement handles forked subgraphs correctly.
    nc.insert_act_table_loads = lambda: None
    nc.compile()
    return nc


def _prep_x(x):
    """[batch, in_f] fp32 -> bf16 tiles with x_t[..., pi, ..., bi] =
    x[bt*128 + bi, po*128 + pi]:
      xg  [P, KB, GROUP, P]      (group tiles 0-7, k-chunk-major)
      xea [P, HALF, EXTRA, P]    (extras 8-11, k 0..15)
      xeb [P, HALF, EXTRA, P]    (extras 8-11, k 16..31)
      xs  [BT-NGRP, P, KB, P]    (stream tiles, tile-major)
    """
    batch, in_f = x.shape
    KB = in_f // P
    HALF = KB // 2
    BT = batch // P
    xbf = x.astype(ml_dtypes.bfloat16)
    xbf = xbf.reshape(BT, P, KB, P)  # [bt, bi, po, pi]
    xt = xbf.transpose(0, 3, 2, 1)  # [bt, pi, po, bi]
    xg = np.ascontiguousarray(xt[:GROUP].transpose(1, 2, 0, 3))  # [pi, po, bt, bi]
    xe = xt[GROUP:NGRP].transpose(1, 2, 0, 3)  # [pi, po, e, bi]
    xea = np.ascontiguousarray(xe[:, :HALF])
    xeb = np.ascontiguousarray(xe[:, HALF:])
    xs = np.ascontiguousarray(xt[NGRP:])
    return xg, xea, xeb, xs


def _tile_w(w, dtype):
    """[o_sh, in_f] -> tiled [KB, 128, o_sh] with w_t[k, pi, o] = w[o, k*128 + pi]."""
    o_sh, in_f = w.shape
    return np.ascontiguousarray(w.T.reshape(in_f // P, P, o_sh)).astype(dtype)


def _prep_wpk(wmu, wrho, weps):
    """Pack mu (bf16), eps (bf16), rho (fp16 bits viewed as bf16) into one
    bf16-typed [KB, 128, 3*o_sh] tensor — one DMA per K-block."""
    mu = _tile_w(wmu, ml_dtypes.bfloat16)
    eps = _tile_w(weps, ml_dtypes.bfloat16)
    rho = _tile_w(wrho, np.float16).view(ml_dtypes.bfloat16)
    return np.ascontiguousarray(np.concatenate([mu, eps, rho], axis=2))


def make_in_maps(x, weight_mu, weight_rho, bias_mu, bias_rho, weight_eps, bias_eps):
    o_sh = OUT_F // N_CORES
    xg, xea, xeb, xs = _prep_x(np.asarray(x, dtype=np.float32))
    wmu = np.asarray(weight_mu, dtype=np.float32)
    wrho = np.asarray(weight_rho, dtype=np.float32)
    weps = np.asarray(weight_eps, dtype=np.float32)
    bpk = np.stack(
        [
            np.asarray(bias_mu, dtype=np.float32),
            np.asarray(bias_rho, dtype=np.float32),
            np.asarray(bias_eps, dtype=np.float32),
        ]
    )  # [3, OUT_F]

    in_maps = []
    for c in range(N_CORES):
        rs = slice(c * o_sh, (c + 1) * o_sh)
        in_maps.append(
            {
                "x_g": xg,
                "x_ea": xea,
                "x_eb": xeb,
                "x_s": xs,
                "wpk_t": _prep_wpk(wmu[rs], wrho[rs], weps[rs]),
                "bias_pk": np.ascontiguousarray(bpk[:, rs].reshape(1, -1)),
            }
        )
    return in_maps


def kernel(x, weight_mu, weight_rho, bias_mu, bias_rho, weight_eps, bias_eps):
    o_sh = OUT_F // N_CORES
    key = (x.shape, o_sh)
    if key not in _NC_CACHE:
        _NC_CACHE[key] = build_nc(x.shape[0], x.shape[1], o_sh)
    nc = _NC_CACHE[key]

    in_maps = make_in_maps(
        x, weight_mu, weight_rho, bias_mu, bias_rho, weight_eps, bias_eps
    )
    res = run_bass_kernel_spmd(nc, in_maps, core_ids=list(range(N_CORES)))
    return np.concatenate([res.results[c]["y"] for c in range(N_CORES)], axis=1)


# revision 12
# speedup vs baseline: 1.0526x; 1.0526x over previous
"""BayesianLinear (y = x @ (mu + softplus(rho) * eps).T + bias) on 8 TRN2 cores.

Column-parallel sharding: each core owns OUT_F/8 = 512 output features.

Host-side prep is pure layout/precision staging (no reference math):
  - x is cast to bf16 and pre-tiled into the SBUF layout the TensorEngine
    needs for its stationary operand. Group tiles (0-7) are laid out
    k-chunk-major so ONE DMA delivers a k-chunk of all 8 tiles with
    fully-contiguous per-partition runs; extras (8-11) ship as two
    half-K blocks; stream tiles (12-63) are tile-major (1MB DMAs).
  - weight mu/rho/eps shards are transposed to [in_f, o_sh] and packed
    per single K-block into one bf16-typed tensor (mu bf16 | eps bf16 |
    rho fp16-bits): 32 DMAs of 393KB in k order on the sync queue, so
    the first W^T block is constructible ~4us after queue start. rho
    ships fp16 because softplus amplifies its quantization ~3x.

Device per core:
  1. sync queue: bias row (6KB), wpk singles k=0..31, then stream x
     tiles (5-deep prefetch ring). scalar queue: group k-chunks and
     extras halves (ordered by consumption time), then y writes. The
     two HWDGE queues round-robin at packet level so weight and x
     delivery progress together; y rides scalar where a trigger's
     eviction-wait cannot head-of-line block stream-x prefetch.
  2. W^T construction per K-block: single-op Softplus(rho) on ACT
     (bf16 out), mul(eps)/add(mu) on DVE at 2x bf16 rate into the
     resident W^T tile [128, 32, 512]. The scalar FIFO interleaves
     x-chunk triggers between softplus ops; the DVE FIFO interleaves
     construction with partial-eviction ops in consumption order
     (wt0-15, partsA, wt16-23/partsA2, wt24-31, tails) so PSUM banks
     free on time.
  3. PE program: warmup K=1 matmuls (HAM ramp + cover of first-block
     latency), bias broadcast (ones.T @ bias_bf), then split-K: tiles
     0-7 accumulate k 0..15 k-interleaved (paced against delivery),
     park bias-pre-added partials in SBUF as bf16; extras 8-11 stream
     first halves at full speed off the resident W^T; both sets finish
     k 16..31. Remaining 52 tiles stream one PSUM bank each; DVE fuses
     eviction with the partial/bias add; y out on the scalar queue.
"""

import numpy as np
import ml_dtypes

import concourse.bacc as bacc
import concourse.mybir as mybir
import concourse.tile as tile
from concourse.bass_utils import run_bass_kernel_spmd

BATCH = 8192
IN_F = 4096
OUT_F = 4096
N_CORES = 8
P = 128

GROUP = 8
EXTRA = 4
NGRP = GROUP + EXTRA
# k-chunk sizes for the group-of-8 delivery: 4-block chunks (1MB DMAs)
# interleaved with the matching pk singles on ONE queue, so delivery
# order equals consumption order.
CHUNKS = [4] * 8

_NC_CACHE = {}


def build_nc(batch=BATCH, in_f=IN_F, o_sh=OUT_F // N_CORES):
    KB = in_f // P  # K-blocks of 128 along the contraction dim
    BT = batch // P  # 128-row output tiles
    HALF = KB // 2

    nc = bacc.Bacc(
        "TRN2",
        target_bir_lowering=False,
        debug=False,
        enable_asserts=False,
        num_devices=N_CORES,
    )
    bf16 = mybir.dt.bfloat16
    f16 = mybir.dt.float16
    f32 = mybir.dt.float32

    cb = [0]
    for c in CHUNKS:
        cb.append(cb[-1] + c)
    assert cb[-1] == KB and HALF in cb
    NCH = len(CHUNKS)

    xg = nc.declare_dram_parameter("x_g", [P, KB, GROUP, P], bf16, isOutput=False)
    xea = nc.declare_dram_parameter("x_ea", [P, HALF, EXTRA, P], bf16, isOutput=False)
    xeb = nc.declare_dram_parameter("x_eb", [P, HALF, EXTRA, P], bf16, isOutput=False)
    xs = nc.declare_dram_parameter("x_s", [BT - NGRP, P, KB, P], bf16, isOutput=False)
    wpk = nc.declare_dram_parameter("wpk_t", [KB, P, 3 * o_sh], bf16, isOutput=False)
    bpk = nc.declare_dram_parameter("bias_pk", [1, 3 * o_sh], f32, isOutput=False)
    y = nc.declare_dram_parameter("y", [batch, o_sh], f32, isOutput=True)

    act_exp = mybir.ActivationFunctionType.Exp
    act_ln = mybir.ActivationFunctionType.Ln

    N_WARM = 17
    RING = 3  # pk/sp construction ring depth

    with tile.TileContext(nc) as tc:
        with (
            tc.tile_pool(name="const", bufs=1) as const,
            tc.tile_pool(name="wcons", bufs=RING) as wcons,
            tc.tile_pool(name="xin", bufs=4) as xin,
            tc.tile_pool(name="part", bufs=NGRP) as part,
            tc.tile_pool(name="yout", bufs=2) as yout,
            tc.tile_pool(name="psum", bufs=8, space="PSUM") as psum_pool,
        ):
            bias_sb = const.tile([P, o_sh], f32, tag="bias_sb")
            bias_bf = const.tile([1, o_sh], bf16, tag="bias_bf")
            ones = const.tile([1, P], bf16, tag="ones")
            nc.vector.memset(ones[:], 1.0)
            wones = const.tile([1, o_sh], bf16, tag="wones")
            nc.vector.memset(wones[:], 1.0)

            # Bias inputs ride the sync queue ahead of everything (6 KiB,
            # one packed partition-0 [1, 3*o_sh] DMA: mu | rho | eps).
            b_all = const.tile([1, 3 * o_sh], f32, tag="b_all")
            nc.sync.dma_start(out=b_all[:], in_=bpk[:])
            b_mu = b_all[:, 0:o_sh]
            b_rho = b_all[:, o_sh : 2 * o_sh]
            b_eps = b_all[:, 2 * o_sh : 3 * o_sh]
            b_sp = const.tile([1, o_sh], f32, tag="b_sp")
            nc.scalar.activation(b_sp[:], b_rho, act_exp)
            nc.scalar.activation(b_sp[:], b_sp[:], act_ln, bias=1.0)
            nc.vector.tensor_mul(out=b_sp[:], in0=b_sp[:], in1=b_eps)
            nc.vector.tensor_add(out=bias_bf[:], in0=b_sp[:], in1=b_mu)

            # PE warmup + bias broadcast emitted early so the DVE-side
            # bias_sb eviction precedes all construction ops in the DVE
            # FIFO (partials add bias_sb at pass-A end).
            warm_ps = psum_pool.tile([P, o_sh], f32, tag="ps", name="warm_ps")
            for w in range(N_WARM):
                nc.tensor.matmul(warm_ps[:], lhsT=ones[:], rhs=wones[:])
            bias_ps = psum_pool.tile([P, o_sh], f32, tag="ps", name="bias_ps")
            nc.tensor.matmul(bias_ps[:], lhsT=ones[:], rhs=bias_bf[:])
            nc.vector.tensor_copy(out=bias_sb[:], in_=bias_ps[:])

            WT_bf = const.tile([P, KB, o_sh], bf16, tag="WT_bf")

            # Group x chunk tiles (k-chunk-major, all 8 tiles per chunk)
            xg_sb = [
                const.tile(
                    [P, CHUNKS[c], GROUP, P], bf16, tag=f"xg_c{c}", name=f"xg_c{c}"
                )
                for c in range(NCH)
            ]
            xe_sb = [
                const.tile(
                    [P, HALF, EXTRA, P], bf16, tag=f"xe_h{h}", name=f"xe_h{h}"
                )
                for h in range(2)
            ]

            # ---- sync-queue DMA program in consumption order: pk
            # singles for a 4-block run, then the x chunk covering those
            # k's; extras halves placed just ahead of their passes. One
            # queue means no SDMA round-robin unfairness — delivery
            # order IS this order. The softplus (scalar) ops are emitted
            # alongside so their queue positions track pk arrival.
            pks = []
            sps = []

            def emit_pk(k):
                pk = wcons.tile([P, 3 * o_sh], bf16, tag="pk", name=f"pk{k}")
                nc.sync.dma_start(out=pk[:], in_=wpk[k])
                pks.append(pk)
                # softplus(rho) = ln(1 + exp(rho)); Exp+Ln share one ACT
                # table set (natural_log_exp_and_others).
                rho_t = pk[:, 2 * o_sh : 3 * o_sh].bitcast(f16)
                sp_f = wcons.tile([P, o_sh], f16, tag="spf", name=f"spf{k}")
                sp_t = wcons.tile([P, o_sh], bf16, tag="sp", name=f"sp{k}")
                nc.scalar.activation(sp_f[:], rho_t[:], act_exp)
                nc.scalar.activation(sp_t[:], sp_f[:], act_ln, bias=1.0)
                sps.append(sp_t)

            def emit_wt(k):
                pk = pks[k]
                mu_t = pk[:, 0:o_sh]
                eps_t = pk[:, o_sh : 2 * o_sh]
                sp_t = sps[k]
                nc.vector.tensor_mul(out=sp_t[:], in0=sp_t[:], in1=eps_t[:])
                nc.vector.tensor_add(out=WT_bf[:, k, :], in0=sp_t[:], in1=mu_t[:])

            for c in range(NCH):
                for k in range(cb[c], cb[c + 1]):
                    emit_pk(k)
                nc.sync.dma_start(out=xg_sb[c][:], in_=xg[:, cb[c] : cb[c + 1]])
                if cb[c + 1] == HALF:
                    nc.sync.dma_start(out=xe_sb[0][:], in_=xea[:])
                if cb[c + 1] == KB:
                    nc.sync.dma_start(out=xe_sb[1][:], in_=xeb[:])

            # ---- DVE construction for the first half (pass-A weights)
            for k in range(HALF):
                emit_wt(k)

            def glhs(i, k):
                """lhsT AP for group tile i (0..NGRP-1), k-block k."""
                if i < GROUP:
                    c = next(
                        ci for ci in range(NCH) if cb[ci] <= k < cb[ci + 1]
                    )
                    return xg_sb[c][:, k - cb[c], i, :]
                return xe_sb[k // HALF][:, k % HALF, i - GROUP, :]

            # Split-K pass A: tiles 0-7, k 0..15, k-interleaved.
            pss = [
                psum_pool.tile([P, o_sh], f32, tag="ps", name=f"ps_a{bt}")
                for bt in range(GROUP)
            ]
            for k in range(HALF):
                for i in range(GROUP):
                    nc.tensor.matmul(
                        pss[i][:],
                        lhsT=glhs(i, k),
                        rhs=WT_bf[:, k, :],
                        start=(k == 0),
                        stop=(k == HALF - 1),
                    )
            parts = []
            for i in range(GROUP):
                pa = part.tile([P, o_sh], bf16, tag="pA", name=f"pA_{i}")
                nc.vector.tensor_add(out=pa[:], in0=pss[i][:], in1=bias_sb[:])
                parts.append(pa)
            # A2: extras' first halves at full speed (W^T 0..HALF
            # resident); second-half constructions interleave on DVE so
            # each partsA2 eviction stays unblocked in the FIFO.
            wt_next = HALF
            for e in range(GROUP, NGRP):
                emit_wt(wt_next)
                emit_wt(wt_next + 1)
                wt_next += 2
                ps = psum_pool.tile([P, o_sh], f32, tag="ps", name=f"ps_a{e}")
                for k in range(HALF):
                    nc.tensor.matmul(
                        ps[:],
                        lhsT=glhs(e, k),
                        rhs=WT_bf[:, k, :],
                        start=(k == 0),
                        stop=(k == HALF - 1),
                    )
                pa = part.tile([P, o_sh], bf16, tag="pA", name=f"pA_{e}")
                nc.vector.tensor_add(out=pa[:], in0=ps[:], in1=bias_sb[:])
                parts.append(pa)
            for k in range(wt_next, KB):
                emit_wt(k)
            # B: tiles 0-7 second halves (k-interleaved)
            psb = [
                psum_pool.tile([P, o_sh], f32, tag="ps", name=f"ps_b{bt}")
                for bt in range(GROUP)
            ]
            for k in range(HALF, KB):
                for i in range(GROUP):
                    nc.tensor.matmul(
                        psb[i][:],
                        lhsT=glhs(i, k),
                        rhs=WT_bf[:, k, :],
                        start=(k == HALF),
                        stop=(k == KB - 1),
                    )

            def split_tail(ps, pa, bt):
                y_sb = yout.tile([P, o_sh], f32, tag="y_sb")
                nc.vector.tensor_add(out=y_sb[:], in0=ps[:], in1=pa[:])
                nc.scalar.dma_start(out=y[bt * P : (bt + 1) * P, :], in_=y_sb[:])

            for i in range(GROUP):
                split_tail(psb[i], parts[i], i)
            # B2: extras' second halves at full speed
            for e in range(GROUP, NGRP):
                ps = psum_pool.tile([P, o_sh], f32, tag="ps", name=f"ps_b{e}")
                for k in range(HALF, KB):
                    nc.tensor.matmul(
                        ps[:],
                        lhsT=glhs(e, k),
                        rhs=WT_bf[:, k, :],
                        start=(k == HALF),
                        stop=(k == KB - 1),
                    )
                split_tail(ps, parts[e], e)

            # ---- remaining tiles stream one PSUM bank each off a
            # 5-deep prefetch ring on the sync queue.
            for bt in range(NGRP, BT):
                xbf_t = xin.tile([P, KB, P], bf16, tag="xT")
                nc.sync.dma_start(out=xbf_t[:], in_=xs[bt - NGRP])
                ps = psum_pool.tile([P, o_sh], f32, tag="ps")
                for k in range(KB):
                    nc.tensor.matmul(
                        ps[:],
                        lhsT=xbf_t[:, k, :],
                        rhs=WT_bf[:, k, :],
                        start=(k == 0),
                        stop=(k == KB - 1),
                    )
                y_sb = yout.tile([P, o_sh], f32, tag="y_sb")
                nc.vector.tensor_add(out=y_sb[:], in0=ps[:], in1=bias_sb[:])
                nc.scalar.dma_start(out=y[bt * P : (bt + 1) * P, :], in_=y_sb[:])

    # Skip bacc's pre-placed InstLoadActFuncSet: on large graphs walrus's
    # parallel-pass fork can separate the hoisted load from its activations
    # ("No Act func set exist for this instruction"); walrus's own lower_act
    # placement handles forked subgraphs correctly.
    nc.insert_act_table_loads = lambda: None
    nc.compile()
    return nc


def _prep_x(x):
    """[batch, in_f] fp32 -> bf16 tiles with x_t[..., pi, ..., bi] =
    x[bt*128 + bi, po*128 + pi]:
      xg  [P, KB, GROUP, P]      (group tiles 0-7, k-chunk-major)
      xea [P, HALF, EXTRA, P]    (extras 8-11, k 0..15)
      xeb [P, HALF, EXTRA, P]    (extras 8-11, k 16..31)
      xs  [BT-NGRP, P, KB, P]    (stream tiles, tile-major)
    """
    batch, in_f = x.shape
    KB = in_f // P
    HALF = KB // 2
    BT = batch // P
    xbf = x.astype(ml_dtypes.bfloat16)
    xbf = xbf.reshape(BT, P, KB, P)  # [bt, bi, po, pi]
    xt = xbf.transpose(0, 3, 2, 1)  # [bt, pi, po, bi]
    xg = np.ascontiguousarray(xt[:GROUP].transpose(1, 2, 0, 3))  # [pi, po, bt, bi]
    xe = xt[GROUP:NGRP].transpose(1, 2, 0, 3)  # [pi, po, e, bi]
    xea = np.ascontiguousarray(xe[:, :HALF])
    xeb = np.ascontiguousarray(xe[:, HALF:])
    xs = np.ascontiguousarray(xt[NGRP:])
    return xg, xea, xeb, xs


def _tile_w(w, dtype):
    """[o_sh, in_f] -> tiled [KB, 128, o_sh] with w_t[k, pi, o] = w[o, k*128 + pi]."""
    o_sh, in_f = w.shape
    return np.ascontiguousarray(w.T.reshape(in_f // P, P, o_sh)).astype(dtype)


def _prep_wpk(wmu, wrho, weps):
    """Pack mu (bf16), eps (bf16), rho (fp16 bits viewed as bf16) into one
    bf16-typed [KB, 128, 3*o_sh] tensor — one DMA per K-block."""
    mu = _tile_w(wmu, ml_dtypes.bfloat16)
    eps = _tile_w(weps, ml_dtypes.bfloat16)
    rho = _tile_w(wrho, np.float16).view(ml_dtypes.bfloat16)
    return np.ascontiguousarray(np.concatenate([mu, eps, rho], axis=2))


def make_in_maps(x, weight_mu, weight_rho, bias_mu, bias_rho, weight_eps, bias_eps):
    o_sh = OUT_F // N_CORES
    xg, xea, xeb, xs = _prep_x(np.asarray(x, dtype=np.float32))
    wmu = np.asarray(weight_mu, dtype=np.float32)
    wrho = np.asarray(weight_rho, dtype=np.float32)
    weps = np.asarray(weight_eps, dtype=np.float32)
    bpk = np.stack(
        [
            np.asarray(bias_mu, dtype=np.float32),
            np.asarray(bias_rho, dtype=np.float32),
            np.asarray(bias_eps, dtype=np.float32),
        ]
    )  # [3, OUT_F]

    in_maps = []
    for c in range(N_CORES):
        rs = slice(c * o_sh, (c + 1) * o_sh)
        in_maps.append(
            {
                "x_g": xg,
                "x_ea": xea,
                "x_eb": xeb,
                "x_s": xs,
                "wpk_t": _prep_wpk(wmu[rs], wrho[rs], weps[rs]),
                "bias_pk": np.ascontiguousarray(bpk[:, rs].reshape(1, -1)),
            }
        )
    return in_maps


def kernel(x, weight_mu, weight_rho, bias_mu, bias_rho, weight_eps, bias_eps):
    o_sh = OUT_F // N_CORES
    key = (x.shape, o_sh)
    if key not in _NC_CACHE:
        _NC_CACHE[key] = build_nc(x.shape[0], x.shape[1], o_sh)
    nc = _NC_CACHE[key]

    in_maps = make_in_maps(
        x, weight_mu, weight_rho, bias_mu, bias_rho, weight_eps, bias_eps
    )
    res = run_bass_kernel_spmd(nc, in_maps, core_ids=list(range(N_CORES)))
    return np.concatenate([res.results[c]["y"] for c in range(N_CORES)], axis=1)
